# revision 4
# baseline (speedup 1.0000x reference)
"""Causal self-attention (B=2, T=2048, C=1024, H=16) on 8 TRN2 NeuronCores.

Sharding: core c handles batch b = c//4 and heads 4*(c%4) .. 4*(c%4)+3
(data-parallel over B, tensor-parallel over heads; full K/V for its heads
is computed locally from the core's QKV projection slice).

Per-core dataflow (all matmuls in float32r = full-rate TF32-like mode):
  - host passes xT = x[b].T [C,T], wqk = [Wq_h|Wk_h]^T [C,512],
    wv = [Wv_h0|0|Wv_h1|0|...]^T [C,260] (65-wide blocks, last col zero)
  - qT/kT [64,T] per head via projection matmuls (contraction c on partitions)
  - v [t,130] per head-pair, with a ones column appended per head (65th col)
  - S^T tiles [k=128, q=512] = kT.T @ qT ; exp on ScalarE (scale=1/8 fused)
  - causal mask on diagonal tiles via gpsimd affine_select (fill 0 after exp)
  - y^T [65, 512] += V'.T @ P^T accumulated over k-tiles; row 64 = softmax
    denominators (from the ones column)
  - normalize: reciprocal_approx_fast on the sums row, partition_broadcast,
    multiply; DMA y^T[h] [64, T] out; host transposes/concats heads.
"""

import os
import sys
import types
import numpy as np

import concourse.bass as bass
import concourse.mybir as mybir
import concourse.tile as tile
from concourse import bacc
from concourse.bass_utils import run_bass_kernel_spmd

B, T, C, H = 2, 2048, 1024, 16
D = 64
NCORES = 8
HPC = 4          # heads per core
NQB = 4          # q blocks of 512
QB = 512
KT = 128         # k tile
F32 = mybir.dt.float32
F32R = mybir.dt.float32r
EXP = mybir.ActivationFunctionType.Exp
MULT = mybir.AluOpType.mult
IS_GE = mybir.AluOpType.is_ge


def _install_profhook():
    """Register the NTFF profile hook shim so BASS_TRACE=1 works; harmless
    no-op (graceful trace skip) when the axon .so lacks profiling."""
    if "antenv.axon_hooks" not in sys.modules:
        mod = types.ModuleType("antenv.axon_hooks")
        mod._hook = None
        mod.set_axon_ntff_profile_hook = lambda h: setattr(mod, "_hook", h)
        mod.get_axon_ntff_profile_hook = lambda: mod._hook
        sys.modules["antenv.axon_hooks"] = mod
        try:
            import antenv
            antenv.axon_hooks = mod
        except ImportError:
            pass
    try:
        from trn_agent_boot.trn_boot import _ntff_profile_via_ctypes
        sys.modules["antenv.axon_hooks"].set_axon_ntff_profile_hook(
            _ntff_profile_via_ctypes("/opt/axon/libaxon_pjrt.so")
        )
        import concourse.bass_utils as bu
        bu.upload_artifacts = lambda tmpdir: tmpdir
    except Exception:
        pass


_install_profhook()

_NC = None


def _build():
    nc = bacc.Bacc("TRN2", target_bir_lowering=False, debug=False,
                   num_devices=NCORES)
    xT_d = nc.declare_dram_parameter("xT", [C, T], F32R, isOutput=False)
    wqk_d = nc.declare_dram_parameter("wqk", [C, 2 * HPC * D], F32R,
                                      isOutput=False)
    wv_d = nc.declare_dram_parameter("wv", [C, HPC * 65], F32R,
                                     isOutput=False)
    y_d = nc.declare_dram_parameter("y", [HPC, D, T], F32, isOutput=True)

    from contextlib import ExitStack
    with tile.TileContext(nc) as tc, ExitStack() as ctx:
        sb = ctx.enter_context(tc.tile_pool(name="sb", bufs=1))
        pp = ctx.enter_context(tc.tile_pool(name="pp", bufs=3))
        yp = ctx.enter_context(tc.tile_pool(name="yp", bufs=3))
        psp = ctx.enter_context(tc.tile_pool(name="psp", bufs=2, space="PSUM"))
        pss = ctx.enter_context(tc.tile_pool(name="pss", bufs=1, space="PSUM"))
        psy = ctx.enter_context(tc.tile_pool(name="psy", bufs=1, space="PSUM"))

        xTs = [sb.tile([128, T], F32R, name=f"xT{i}") for i in range(8)]
        wqks = [sb.tile([128, 512], F32R, name=f"wqk{i}") for i in range(8)]
        wvs = [sb.tile([128, 260], F32R, name=f"wv{i}") for i in range(8)]
        qs = [sb.tile([128, T], F32R, name=f"q{p}") for p in range(2)]
        ks = [sb.tile([128, T], F32R, name=f"k{p}") for p in range(2)]
        vs = [[sb.tile([128, 130], F32R, name=f"v{p}_{t}") for t in range(16)]
              for p in range(2)]
        ones2 = sb.tile([128, 2], F32, name="ones2")
        nc.gpsimd.memset(ones2[:], 1.0)

        for i in range(8):
            nc.sync.dma_start(xTs[i][:], xT_d.ap()[i * 128:(i + 1) * 128, :])
            nc.sync.dma_start(wqks[i][:], wqk_d.ap()[i * 128:(i + 1) * 128, :])
            nc.sync.dma_start(wvs[i][:], wv_d.ap()[i * 128:(i + 1) * 128, :])

        def proj_ops(p):
            """Projection op closures for head-pair p (local heads 2p, 2p+1)."""
            ops = []
            for ft, dst in ((p, qs[p]), (2 + p, ks[p])):
                for tb in range(4):
                    def emit(ft=ft, dst=dst, tb=tb):
                        mm = psp.tile([128, 512], F32,
                                      name=f"pqk{p}_{ft}_{tb}", tag="pmm")
                        for c in range(8):
                            nc.tensor.matmul(
                                mm[:],
                                wqks[c][:, ft * 128:(ft + 1) * 128],
                                xTs[c][:, tb * 512:(tb + 1) * 512],
                                start=(c == 0), stop=(c == 7))
                        nc.vector.tensor_copy(
                            dst[:, tb * 512:(tb + 1) * 512], mm[:])
                    ops.append(emit)
            for tt in range(16):
                def emit(tt=tt):
                    mmv = psp.tile([128, 130], F32,
                                   name=f"pv{p}_{tt}", tag="pmm")
                    for c in range(8):
                        nc.tensor.matmul(
                            mmv[:],
                            xTs[c][:, tt * 128:(tt + 1) * 128],
                            wvs[c][:, p * 130:(p + 1) * 130],
                            start=(c == 0), stop=(c == 7))
                    nc.vector.tensor_copy(vs[p][tt][:], mmv[:])
                    nc.vector.tensor_copy(vs[p][tt][:, 64:130:65], ones2[:])
                ops.append(emit)
            return ops

        def attn_ops(p):
            """Attention op closures for head-pair p: 20 chunk emitters."""
            ops = []
            for j in range(NQB):
                nkt = 4 * (j + 1)
                state = {}
                for cc in range(nkt // 2):
                    def emit(p=p, j=j, cc=cc, state=state, nkt=nkt):
                        if cc == 0:
                            state["ye"] = psy.tile([65, 512], F32,
                                                   name=f"ye{p}_{j}", tag="ye")
                            state["yo"] = psy.tile([65, 512], F32,
                                                   name=f"yo{p}_{j}", tag="yo")
                        kk0, kk1 = 2 * cc, 2 * cc + 1
                        s = pss.tile([128, 2048], F32,
                                     name=f"s{p}_{j}_{cc}", tag="s")
                        qsl = (j * 512, (j + 1) * 512)
                        # quarters: [kk0,e | kk1,e | kk0,o | kk1,o]
                        for qi, (kk, lo, hi) in enumerate(
                                ((kk0, 0, 64), (kk0, 64, 128),
                                 (kk1, 0, 64), (kk1, 64, 128))):
                            col = (0, 1024, 512, 1536)[qi]
                            nc.tensor.matmul(
                                s[:, col:col + 512],
                                ks[p][lo:hi, kk * 128:(kk + 1) * 128],
                                qs[p][lo:hi, qsl[0]:qsl[1]],
                                start=True, stop=True)
                        pt = pp.tile([128, 2048], F32R,
                                     name=f"pt{p}_{j}_{cc}", tag="pt")
                        nc.scalar.activation(pt[:], s[:], EXP, scale=0.125)
                        # causal mask on diagonal k-tiles (keep where q >= k)
                        for kk, cols in ((kk0, (0, 1024)), (kk1, (512, 1536))):
                            if kk >= 4 * j:
                                for col in cols:
                                    nc.gpsimd.affine_select(
                                        pt[:, col:col + 512],
                                        pt[:, col:col + 512],
                                        pattern=[[1, 512]],
                                        compare_op=IS_GE, fill=0.0,
                                        base=512 * j - 128 * kk,
                                        channel_multiplier=-1)
                        first = (cc == 0)
                        last = (cc == nkt // 2 - 1)
                        # PV accumulation (ones column -> row 64 = denominators)
                        nc.tensor.matmul(state["ye"][:],
                                         vs[p][kk0][:, 0:65], pt[:, 0:512],
                                         start=first, stop=False)
                        nc.tensor.matmul(state["yo"][:],
                                         vs[p][kk0][:, 65:130], pt[:, 1024:1536],
                                         start=first, stop=False)
                        nc.tensor.matmul(state["ye"][:],
                                         vs[p][kk1][:, 0:65], pt[:, 512:1024],
                                         start=False, stop=last)
                        nc.tensor.matmul(state["yo"][:],
                                         vs[p][kk1][:, 65:130], pt[:, 1536:2048],
                                         start=False, stop=last)
                        if last:
                            for h01, key in ((0, "ye"), (1, "yo")):
                                ysb = yp.tile([65, 512], F32,
                                              name=f"ysb{p}_{j}_{h01}",
                                              tag="ysb")
                                nc.vector.tensor_copy(ysb[:], state[key][:])
                                ssb = yp.tile([1, 512], F32,
                                              name=f"ssb{p}_{j}_{h01}",
                                              tag="ssb")
                                nc.vector.tensor_copy(ssb[:], ysb[64:65, :])
                                rsb = yp.tile([1, 512], F32,
                                              name=f"rsb{p}_{j}_{h01}",
                                              tag="rsb")
                                nc.vector.reciprocal_approx_fast(
                                    out=rsb[:], in_=ssb[:])
                                bsb = yp.tile([64, 512], F32,
                                              name=f"bsb{p}_{j}_{h01}",
                                              tag="bsb")
                                nc.gpsimd.partition_broadcast(
                                    bsb[:], rsb[:], channels=64)
                                yn = yp.tile([64, 512], F32,
                                             name=f"yn{p}_{j}_{h01}", tag="yn")
                                nc.vector.tensor_tensor(
                                    yn[:], ysb[0:64, :], bsb[:], op=MULT)
                                nc.sync.dma_start(
                                    y_d.ap()[2 * p + h01, :,
                                             j * 512:(j + 1) * 512],
                                    yn[:])
                    ops.append(emit)
            return ops

        # program order: proj0; attn0 with proj1 interleaved; attn1
        for op in proj_ops(0):
            op()
        a0 = attn_ops(0)
        p1 = proj_ops(1)
        k = 0
        for i, op in enumerate(a0):
            op()
            tgt = (i + 1) * len(p1) // len(a0)
            while k < tgt:
                p1[k]()
                k += 1
        for op in attn_ops(1):
            op()

    nc.compile()
    return nc


def _get_nc():
    global _NC
    if _NC is None:
        _NC = _build()
    return _NC


def _make_in_maps(x, W_attn):
    x = np.asarray(x, dtype=np.float32)
    W = np.asarray(W_attn, dtype=np.float32)
    wq, wk, wv = W[0:C], W[C:2 * C], W[2 * C:3 * C]
    in_maps = []
    for c in range(NCORES):
        b, g = c // 4, c % 4
        heads = [HPC * g + i for i in range(HPC)]
        xTb = np.ascontiguousarray(x[b].T)
        qrows = np.concatenate([wq[D * h:D * h + D] for h in heads], axis=0)
        krows = np.concatenate([wk[D * h:D * h + D] for h in heads], axis=0)
        wqk_np = np.ascontiguousarray(np.concatenate([qrows, krows], 0).T)
        wv_np = np.zeros((C, HPC * 65), np.float32)
        for i, h in enumerate(heads):
            wv_np[:, 65 * i:65 * i + D] = wv[D * h:D * h + D].T
        in_maps.append({"xT": xTb, "wqk": wqk_np, "wv": wv_np})
    return in_maps


def _execute(in_maps, trace=False):
    return run_bass_kernel_spmd(_get_nc(), in_maps,
                                core_ids=list(range(NCORES)), trace=trace)


def _assemble(results):
    y = np.empty((B, T, C), np.float32)
    for c in range(NCORES):
        b, g = c // 4, c % 4
        yc = results[c]["y"]
        for i in range(HPC):
            h = HPC * g + i
            y[b, :, D * h:D * h + D] = yc[i].T
    return y


def kernel(x, W_attn):
    res = _execute(_make_in_maps(x, W_attn), trace=False)
    return _assemble(res.results)


# revision 5
# speedup vs baseline: 1.3178x; 1.3178x over previous
"""Causal self-attention (B=2, T=2048, C=1024, H=16) on 8 TRN2 NeuronCores.

Sharding: core c handles batch b = c//4 and heads 4*(c%4) .. 4*(c%4)+3
(data-parallel over B, tensor-parallel over heads; full K/V for its heads
is computed locally from the core's QKV projection slice).

Per-core dataflow (all matmuls in float32r = full-rate TF32-like mode):
  - host passes xT = x[b].T [C,T], wqk = [Wq_h|Wk_h]^T [C,512],
    wv = [Wv_h0|0|...|Wv_h3|0]^T [C,260] (65-wide blocks, last col zero)
  - qT/kT [64,T] per head via projection matmuls (contraction c on partitions)
  - v [t,260] with a ones column appended per head (65th of each block)
  - head pairs (2p, 2p+1) share S^T tiles: s [k=128, 1024] = [S_even|S_odd],
    exp on ScalarE (scale=1/8 fused), causal mask on diagonal k-tiles via one
    gpsimd affine_select over a [128,2,512] view
  - y^T [65, 512] += V'.T @ P^T accumulated over k-tiles; row 64 = softmax
    denominators (from the ones column)
  - normalize: reciprocal_approx_fast + gpsimd partition_broadcast + multiply
  - DMA y^T[h] [64, T] out; host transposes/concats heads.
"""

import os
import sys
import types
import numpy as np

import concourse.bass as bass
import concourse.mybir as mybir
import concourse.tile as tile
from concourse import bacc
from concourse.bass_utils import run_bass_kernel_spmd

B, T, C, H = 2, 2048, 1024, 16
D = 64
NCORES = 8
HPC = 4          # heads per core
NQB = 4          # q blocks of 512
QB = 512
F32 = mybir.dt.float32
F32R = mybir.dt.float32r
EXP = mybir.ActivationFunctionType.Exp
MULT = mybir.AluOpType.mult
IS_GE = mybir.AluOpType.is_ge


def _install_profhook():
    """Register the NTFF profile hook shim so BASS_TRACE=1 works; harmless
    no-op (graceful trace skip) when the axon .so lacks profiling."""
    if "antenv.axon_hooks" not in sys.modules:
        mod = types.ModuleType("antenv.axon_hooks")
        mod._hook = None
        mod.set_axon_ntff_profile_hook = lambda h: setattr(mod, "_hook", h)
        mod.get_axon_ntff_profile_hook = lambda: mod._hook
        sys.modules["antenv.axon_hooks"] = mod
        try:
            import antenv
            antenv.axon_hooks = mod
        except ImportError:
            pass
    try:
        from trn_agent_boot.trn_boot import _ntff_profile_via_ctypes
        sys.modules["antenv.axon_hooks"].set_axon_ntff_profile_hook(
            _ntff_profile_via_ctypes("/opt/axon/libaxon_pjrt.so")
        )
        import concourse.bass_utils as bu
        bu.upload_artifacts = lambda tmpdir: tmpdir
    except Exception:
        pass


_install_profhook()

_NC = None


def _build():
    nc = bacc.Bacc("TRN2", target_bir_lowering=False, debug=False,
                   num_devices=NCORES)
    xT_d = nc.declare_dram_parameter("xT", [C, T], F32R, isOutput=False)
    wqk_d = nc.declare_dram_parameter("wqk", [C, 2 * HPC * D], F32R,
                                      isOutput=False)
    wv_d = nc.declare_dram_parameter("wv", [C, HPC * 65], F32R,
                                     isOutput=False)
    y_d = nc.declare_dram_parameter("y", [HPC, D, T], F32, isOutput=True)

    from contextlib import ExitStack
    with tile.TileContext(nc) as tc, ExitStack() as ctx:
        sb = ctx.enter_context(tc.tile_pool(name="sb", bufs=1))
        pp = ctx.enter_context(tc.tile_pool(name="pp", bufs=4))
        yp = ctx.enter_context(tc.tile_pool(name="yp", bufs=3))
        psp = ctx.enter_context(tc.tile_pool(name="psp", bufs=2, space="PSUM"))
        pss = ctx.enter_context(tc.tile_pool(name="pss", bufs=2, space="PSUM"))
        psy = ctx.enter_context(tc.tile_pool(name="psy", bufs=1, space="PSUM"))

        xTs = [sb.tile([128, T], F32R, name=f"xT{i}") for i in range(8)]
        wqks = [sb.tile([128, 512], F32R, name=f"wqk{i}") for i in range(8)]
        wvs = [sb.tile([128, 260], F32R, name=f"wv{i}") for i in range(8)]
        qs = [sb.tile([128, T], F32R, name=f"q{p}") for p in range(2)]
        ks = [sb.tile([128, T], F32R, name=f"k{p}") for p in range(2)]
        vs = [sb.tile([128, 260], F32R, name=f"v_{t}") for t in range(16)]
        ones2 = sb.tile([128, 4], F32, name="ones2")
        nc.gpsimd.memset(ones2[:], 1.0)

        for i in range(8):
            nc.sync.dma_start(xTs[i][:], xT_d.ap()[i * 128:(i + 1) * 128, :])
            nc.sync.dma_start(wqks[i][:], wqk_d.ap()[i * 128:(i + 1) * 128, :])
            nc.sync.dma_start(wvs[i][:], wv_d.ap()[i * 128:(i + 1) * 128, :])

        def v_proj_ops():
            """Combined v projection for all 4 heads: 16 t-tiles, N=260."""
            ops = []
            for tt in range(16):
                def emit(tt=tt):
                    mmv = psp.tile([128, 260], F32, name=f"pv{tt}", tag="pmm")
                    for c in range(8):
                        nc.tensor.matmul(
                            mmv[:],
                            xTs[c][:, tt * 128:(tt + 1) * 128],
                            wvs[c][:],
                            start=(c == 0), stop=(c == 7))
                    nc.vector.tensor_copy(vs[tt][:], mmv[:])
                    nc.vector.tensor_copy(vs[tt][:, 64:260:65], ones2[:])
                ops.append(emit)
            return ops

        def qk_proj_ops(p):
            """qT/kT projection for head-pair p (local heads 2p, 2p+1)."""
            ops = []
            for ft, dst in ((p, qs[p]), (2 + p, ks[p])):
                for tb in range(4):
                    def emit(ft=ft, dst=dst, tb=tb):
                        mm = psp.tile([128, 512], F32,
                                      name=f"pqk{p}_{ft}_{tb}", tag="pmm")
                        for c in range(8):
                            nc.tensor.matmul(
                                mm[:],
                                wqks[c][:, ft * 128:(ft + 1) * 128],
                                xTs[c][:, tb * 512:(tb + 1) * 512],
                                start=(c == 0), stop=(c == 7))
                        nc.vector.tensor_copy(
                            dst[:, tb * 512:(tb + 1) * 512], mm[:])
                    ops.append(emit)
            return ops

        def attn_ops(p):
            """Attention for head-pair p: one chunk per (q-block, k-tile)."""
            ops = []
            for j in range(NQB):
                nkt = 4 * (j + 1)
                state = {}
                for kk in range(nkt):
                    def emit(p=p, j=j, kk=kk, state=state, nkt=nkt):
                        if kk == 0:
                            state["ye"] = psy.tile([65, 512], F32,
                                                   name=f"ye{p}_{j}", tag="ye")
                            state["yo"] = psy.tile([65, 512], F32,
                                                   name=f"yo{p}_{j}", tag="yo")
                        s = pss.tile([128, 1024], F32,
                                     name=f"s{p}_{j}_{kk}", tag="s")
                        jq = (j * 512, (j + 1) * 512)
                        ksl = (kk * 128, (kk + 1) * 128)
                        nc.tensor.matmul(s[:, 0:512],
                                         ks[p][0:64, ksl[0]:ksl[1]],
                                         qs[p][0:64, jq[0]:jq[1]],
                                         start=True, stop=True)
                        nc.tensor.matmul(s[:, 512:1024],
                                         ks[p][64:128, ksl[0]:ksl[1]],
                                         qs[p][64:128, jq[0]:jq[1]],
                                         start=True, stop=True)
                        pt = pp.tile([128, 1024], F32R,
                                     name=f"pt{p}_{j}_{kk}", tag="pt")
                        nc.scalar.activation(pt[:], s[:], EXP, scale=0.125)
                        if kk >= 4 * j:
                            # causal mask both head halves in one op:
                            # [128, 2, 512] view, keep where q >= k
                            v3 = pt[:].rearrange("p (b q) -> p b q", b=2)
                            nc.gpsimd.affine_select(
                                v3, v3,
                                pattern=[[0, 2], [1, 512]],
                                compare_op=IS_GE, fill=0.0,
                                base=512 * j - 128 * kk,
                                channel_multiplier=-1)
                        first, last = (kk == 0), (kk == nkt - 1)
                        nc.tensor.matmul(state["ye"][:],
                                         vs[kk][:, 130 * p:130 * p + 65],
                                         pt[:, 0:512],
                                         start=first, stop=last)
                        nc.tensor.matmul(state["yo"][:],
                                         vs[kk][:, 130 * p + 65:130 * p + 130],
                                         pt[:, 512:1024],
                                         start=first, stop=last)
                        if last:
                            for h01, key in ((0, "ye"), (1, "yo")):
                                ysb = yp.tile([65, 512], F32,
                                              name=f"ysb{p}_{j}_{h01}",
                                              tag="ysb")
                                nc.vector.tensor_copy(ysb[:], state[key][:])
                                ssb = yp.tile([1, 512], F32,
                                              name=f"ssb{p}_{j}_{h01}",
                                              tag="ssb")
                                nc.vector.tensor_copy(ssb[:], ysb[64:65, :])
                                rsb = yp.tile([1, 512], F32,
                                              name=f"rsb{p}_{j}_{h01}",
                                              tag="rsb")
                                nc.vector.reciprocal_approx_fast(
                                    out=rsb[:], in_=ssb[:])
                                bsb = yp.tile([64, 512], F32,
                                              name=f"bsb{p}_{j}_{h01}",
                                              tag="bsb")
                                nc.gpsimd.partition_broadcast(
                                    bsb[:], rsb[:], channels=64)
                                yn = yp.tile([64, 512], F32,
                                             name=f"yn{p}_{j}_{h01}", tag="yn")
                                nc.vector.tensor_tensor(
                                    yn[:], ysb[0:64, :], bsb[:], op=MULT)
                                nc.sync.dma_start(
                                    y_d.ap()[2 * p + h01, :,
                                             j * 512:(j + 1) * 512],
                                    yn[:])
                    ops.append(emit)
            return ops

        # program order: v+qk0; attn0 with qk1 interleaved; attn1
        for op in v_proj_ops():
            op()
        for op in qk_proj_ops(0):
            op()
        a0 = attn_ops(0)
        p1 = qk_proj_ops(1)
        k = 0
        for i, op in enumerate(a0):
            op()
            tgt = (i + 1) * len(p1) // len(a0)
            while k < tgt:
                p1[k]()
                k += 1
        for op in attn_ops(1):
            op()

    nc.compile()
    return nc


def _get_nc():
    global _NC
    if _NC is None:
        _NC = _build()
    return _NC


def _make_in_maps(x, W_attn):
    x = np.asarray(x, dtype=np.float32)
    W = np.asarray(W_attn, dtype=np.float32)
    wq, wk, wv = W[0:C], W[C:2 * C], W[2 * C:3 * C]
    in_maps = []
    for c in range(NCORES):
        b, g = c // 4, c % 4
        heads = [HPC * g + i for i in range(HPC)]
        xTb = np.ascontiguousarray(x[b].T)
        qrows = np.concatenate([wq[D * h:D * h + D] for h in heads], axis=0)
        krows = np.concatenate([wk[D * h:D * h + D] for h in heads], axis=0)
        wqk_np = np.ascontiguousarray(np.concatenate([qrows, krows], 0).T)
        wv_np = np.zeros((C, HPC * 65), np.float32)
        for i, h in enumerate(heads):
            wv_np[:, 65 * i:65 * i + D] = wv[D * h:D * h + D].T
        in_maps.append({"xT": xTb, "wqk": wqk_np, "wv": wv_np})
    return in_maps


def _execute(in_maps, trace=False):
    return run_bass_kernel_spmd(_get_nc(), in_maps,
                                core_ids=list(range(NCORES)), trace=trace)


def _assemble(results):
    y = np.empty((B, T, C), np.float32)
    for c in range(NCORES):
        b, g = c // 4, c % 4
        yc = results[c]["y"]
        for i in range(HPC):
            h = HPC * g + i
            y[b, :, D * h:D * h + D] = yc[i].T
    return y


def kernel(x, W_attn):
    res = _execute(_make_in_maps(x, W_attn), trace=False)
    return _assemble(res.results)


# revision 6
# speedup vs baseline: 1.4024x; 1.0642x over previous
"""Causal self-attention (B=2, T=2048, C=1024, H=16) on 8 TRN2 NeuronCores.

Sharding: core c handles batch b = c//4 and heads 4*(c%4) .. 4*(c%4)+3
(data-parallel over B, tensor-parallel over heads; full K/V for its heads
is computed locally from the core's QKV projection slice).

Per-core dataflow (all matmuls in float32r = full-rate TF32-like mode):
  - host passes xT = x[b].T [C,T], wqk = [Wq_h|Wk_h]^T [C,512],
    wv = [Wv_h0|0|...|Wv_h3|0]^T [C,260] (65-wide blocks, last col zero)
  - qT/kT [64,T] per head via projection matmuls (contraction c on partitions)
  - v [t,260] with a ones column appended per head (65th of each block)
  - head pairs (2p, 2p+1) share S^T tiles: s [k=128, 1024] = [S_even|S_odd],
    exp on ScalarE (scale=1/8 fused), causal mask on diagonal k-tiles via one
    gpsimd affine_select over a [128,2,512] view
  - y^T [65, 512] += V'.T @ P^T accumulated over k-tiles; row 64 = softmax
    denominators (from the ones column)
  - normalize: reciprocal_approx_fast + gpsimd partition_broadcast + multiply
  - DMA y^T[h] [64, T] out; host transposes/concats heads.

Pipelining: inputs are DMA'd in t-block slices and the emission order stages
projection chains immediately ahead of the attention q-blocks that consume
them, so TensorE stays dense from ~10us on and ScalarE (exp) starts early.
"""

import os
import sys
import types
import numpy as np

import concourse.bass as bass
import concourse.mybir as mybir
import concourse.tile as tile
from concourse import bacc
from concourse.bass_utils import run_bass_kernel_spmd

B, T, C, H = 2, 2048, 1024, 16
D = 64
NCORES = 8
HPC = 4          # heads per core
NQB = 4          # q blocks of 512
QB = 512
F32 = mybir.dt.float32
F32R = mybir.dt.float32r
EXP = mybir.ActivationFunctionType.Exp
MULT = mybir.AluOpType.mult
IS_GE = mybir.AluOpType.is_ge


def _install_profhook():
    """Register the NTFF profile hook shim so BASS_TRACE=1 works; harmless
    no-op (graceful trace skip) when the axon .so lacks profiling."""
    if "antenv.axon_hooks" not in sys.modules:
        mod = types.ModuleType("antenv.axon_hooks")
        mod._hook = None
        mod.set_axon_ntff_profile_hook = lambda h: setattr(mod, "_hook", h)
        mod.get_axon_ntff_profile_hook = lambda: mod._hook
        sys.modules["antenv.axon_hooks"] = mod
        try:
            import antenv
            antenv.axon_hooks = mod
        except ImportError:
            pass
    try:
        from trn_agent_boot.trn_boot import _ntff_profile_via_ctypes
        sys.modules["antenv.axon_hooks"].set_axon_ntff_profile_hook(
            _ntff_profile_via_ctypes("/opt/axon/libaxon_pjrt.so")
        )
        import concourse.bass_utils as bu
        bu.upload_artifacts = lambda tmpdir: tmpdir
    except Exception:
        pass


_install_profhook()

_NC = None


def _build():
    nc = bacc.Bacc("TRN2", target_bir_lowering=False, debug=False,
                   num_devices=NCORES)
    xT_d = nc.declare_dram_parameter("xT", [C, T], F32R, isOutput=False)
    wqk_d = nc.declare_dram_parameter("wqk", [C, 2 * HPC * D], F32R,
                                      isOutput=False)
    wv_d = nc.declare_dram_parameter("wv", [C, HPC * 65], F32R,
                                     isOutput=False)
    y_d = nc.declare_dram_parameter("y", [HPC, D, T], F32, isOutput=True)

    from contextlib import ExitStack
    with tile.TileContext(nc) as tc, ExitStack() as ctx:
        sb = ctx.enter_context(tc.tile_pool(name="sb", bufs=1))
        pp = ctx.enter_context(tc.tile_pool(name="pp", bufs=4))
        yp = ctx.enter_context(tc.tile_pool(name="yp", bufs=3))
        psp = ctx.enter_context(tc.tile_pool(name="psp", bufs=2, space="PSUM"))
        pss = ctx.enter_context(tc.tile_pool(name="pss", bufs=2, space="PSUM"))
        psy = ctx.enter_context(tc.tile_pool(name="psy", bufs=1, space="PSUM"))

        # per-(c, t-block) x^T tiles; per-t-block q/k tiles -> fine-grain deps
        xTt = [[sb.tile([128, 512], F32R, name=f"xT{c}_{tb}")
                for tb in range(4)] for c in range(8)]
        wqks = [sb.tile([128, 512], F32R, name=f"wqk{i}") for i in range(8)]
        wvs = [sb.tile([128, 260], F32R, name=f"wv{i}") for i in range(8)]
        qs = [[sb.tile([128, 512], F32R, name=f"q{p}_{tb}") for tb in range(4)]
              for p in range(2)]
        ks = [[sb.tile([128, 512], F32R, name=f"k{p}_{tb}") for tb in range(4)]
              for p in range(2)]
        vs = [sb.tile([128, 260], F32R, name=f"v_{t}") for t in range(16)]
        ones2 = sb.tile([128, 4], F32, name="ones2")
        nc.gpsimd.memset(ones2[:], 1.0)

        # DMA order: wqk, xT t-block 0, wv, xT t-blocks 1..3
        for c in range(8):
            nc.sync.dma_start(wqks[c][:], wqk_d.ap()[c * 128:(c + 1) * 128, :])
        for c in range(8):
            nc.sync.dma_start(xTt[c][0][:],
                              xT_d.ap()[c * 128:(c + 1) * 128, 0:512])
        for c in range(8):
            nc.sync.dma_start(wvs[c][:], wv_d.ap()[c * 128:(c + 1) * 128, :])
        for tb in range(1, 4):
            for c in range(8):
                nc.sync.dma_start(
                    xTt[c][tb][:],
                    xT_d.ap()[c * 128:(c + 1) * 128, tb * 512:(tb + 1) * 512])

        def qk_chain(p, ft_kind, tb):
            """One projection chain: q (ft_kind=0) or k (ft_kind=1) of pair p,
            t-block tb."""
            ft = p if ft_kind == 0 else 2 + p
            dst = (qs if ft_kind == 0 else ks)[p][tb]
            mm = psp.tile([128, 512], F32, name=f"pqk{p}_{ft}_{tb}", tag="pmm")
            for c in range(8):
                nc.tensor.matmul(mm[:],
                                 wqks[c][:, ft * 128:(ft + 1) * 128],
                                 xTt[c][tb][:],
                                 start=(c == 0), stop=(c == 7))
            nc.vector.tensor_copy(dst[:], mm[:])

        def v_chain(tt):
            """Combined v projection for one t-tile (all 4 heads, N=260)."""
            tb, sub = tt // 4, tt % 4
            mmv = psp.tile([128, 260], F32, name=f"pv{tt}", tag="pmm")
            for c in range(8):
                nc.tensor.matmul(mmv[:],
                                 xTt[c][tb][:, sub * 128:(sub + 1) * 128],
                                 wvs[c][:],
                                 start=(c == 0), stop=(c == 7))
            nc.vector.tensor_copy(vs[tt][:], mmv[:])
            nc.vector.tensor_copy(vs[tt][:, 64:260:65], ones2[:])

        def attn_chunks(p, j):
            """Chunk emitters for (pair p, q-block j): one per k-tile."""
            nkt = 4 * (j + 1)
            state = {}
            ops = []
            for kk in range(nkt):
                def emit(p=p, j=j, kk=kk, state=state, nkt=nkt):
                    if kk == 0:
                        state["ye"] = psy.tile([65, 512], F32,
                                               name=f"ye{p}_{j}", tag="ye")
                        state["yo"] = psy.tile([65, 512], F32,
                                               name=f"yo{p}_{j}", tag="yo")
                    s = pss.tile([128, 1024], F32,
                                 name=f"s{p}_{j}_{kk}", tag="s")
                    ktb, ksub = kk // 4, (kk % 4) * 128
                    nc.tensor.matmul(s[:, 0:512],
                                     ks[p][ktb][0:64, ksub:ksub + 128],
                                     qs[p][j][0:64, :],
                                     start=True, stop=True)
                    nc.tensor.matmul(s[:, 512:1024],
                                     ks[p][ktb][64:128, ksub:ksub + 128],
                                     qs[p][j][64:128, :],
                                     start=True, stop=True)
                    pt = pp.tile([128, 1024], F32R,
                                 name=f"pt{p}_{j}_{kk}", tag="pt")
                    nc.scalar.activation(pt[:], s[:], EXP, scale=0.125)
                    if kk >= 4 * j:
                        # causal mask both head halves in one op:
                        # [128, 2, 512] view, keep where q >= k
                        v3 = pt[:].rearrange("p (b q) -> p b q", b=2)
                        nc.gpsimd.affine_select(
                            v3, v3,
                            pattern=[[0, 2], [1, 512]],
                            compare_op=IS_GE, fill=0.0,
                            base=512 * j - 128 * kk,
                            channel_multiplier=-1)
                    first, last = (kk == 0), (kk == nkt - 1)
                    nc.tensor.matmul(state["ye"][:],
                                     vs[kk][:, 130 * p:130 * p + 65],
                                     pt[:, 0:512],
                                     start=first, stop=last)
                    nc.tensor.matmul(state["yo"][:],
                                     vs[kk][:, 130 * p + 65:130 * p + 130],
                                     pt[:, 512:1024],
                                     start=first, stop=last)
                    if last:
                        for h01, key in ((0, "ye"), (1, "yo")):
                            ysb = yp.tile([65, 512], F32,
                                          name=f"ysb{p}_{j}_{h01}", tag="ysb")
                            nc.vector.tensor_copy(ysb[:], state[key][:])
                            ssb = yp.tile([1, 512], F32,
                                          name=f"ssb{p}_{j}_{h01}", tag="ssb")
                            nc.vector.tensor_copy(ssb[:], ysb[64:65, :])
                            rsb = yp.tile([1, 512], F32,
                                          name=f"rsb{p}_{j}_{h01}", tag="rsb")
                            nc.vector.reciprocal_approx_fast(
                                out=rsb[:], in_=ssb[:])
                            bsb = yp.tile([64, 512], F32,
                                          name=f"bsb{p}_{j}_{h01}", tag="bsb")
                            nc.gpsimd.partition_broadcast(
                                bsb[:], rsb[:], channels=64)
                            yn = yp.tile([64, 512], F32,
                                         name=f"yn{p}_{j}_{h01}", tag="yn")
                            nc.vector.tensor_tensor(
                                yn[:], ysb[0:64, :], bsb[:], op=MULT)
                            nc.sync.dma_start(
                                y_d.ap()[2 * p + h01, :,
                                         j * 512:(j + 1) * 512],
                                yn[:])
                ops.append(emit)
            return ops

        # staged emission: per q-block j, pair-0 projections for t-block j,
        # the v tiles it needs, then attn0 q-block j with pair-1 projections
        # interleaved; attn1 runs last (its inputs are all ready by then).
        for j in range(NQB):
            qk_chain(0, 0, j)          # q pair0, t-block j
            qk_chain(0, 1, j)          # k pair0, t-block j
            for tt in range(4 * j, 4 * j + 4):
                v_chain(tt)
            chunks = attn_chunks(0, j)
            p1work = [lambda j=j: qk_chain(1, 0, j),
                      lambda j=j: qk_chain(1, 1, j)]
            k = 0
            for i, op in enumerate(chunks):
                op()
                tgt = (i + 1) * len(p1work) // len(chunks)
                while k < tgt:
                    p1work[k]()
                    k += 1
        for j in range(NQB):
            for op in attn_chunks(1, j):
                op()

    nc.compile()
    return nc


def _get_nc():
    global _NC
    if _NC is None:
        _NC = _build()
    return _NC


def _make_in_maps(x, W_attn):
    x = np.asarray(x, dtype=np.float32)
    W = np.asarray(W_attn, dtype=np.float32)
    wq, wk, wv = W[0:C], W[C:2 * C], W[2 * C:3 * C]
    in_maps = []
    for c in range(NCORES):
        b, g = c // 4, c % 4
        heads = [HPC * g + i for i in range(HPC)]
        xTb = np.ascontiguousarray(x[b].T)
        qrows = np.concatenate([wq[D * h:D * h + D] for h in heads], axis=0)
        krows = np.concatenate([wk[D * h:D * h + D] for h in heads], axis=0)
        wqk_np = np.ascontiguousarray(np.concatenate([qrows, krows], 0).T)
        wv_np = np.zeros((C, HPC * 65), np.float32)
        for i, h in enumerate(heads):
            wv_np[:, 65 * i:65 * i + D] = wv[D * h:D * h + D].T
        in_maps.append({"xT": xTb, "wqk": wqk_np, "wv": wv_np})
    return in_maps


def _execute(in_maps, trace=False):
    return run_bass_kernel_spmd(_get_nc(), in_maps,
                                core_ids=list(range(NCORES)), trace=trace)


def _assemble(results):
    y = np.empty((B, T, C), np.float32)
    for c in range(NCORES):
        b, g = c // 4, c % 4
        yc = results[c]["y"]
        for i in range(HPC):
            h = HPC * g + i
            y[b, :, D * h:D * h + D] = yc[i].T
    return y


def kernel(x, W_attn):
    res = _execute(_make_in_maps(x, W_attn), trace=False)
    return _assemble(res.results)
